# revision 5
# baseline (speedup 1.0000x reference)
"""Trainium2 Bass kernel for nn_NeuronCircuit_15229954031678 (moe_routing).

Math (per batch b, token s):
  h = sum_n cw[s,n] * (x[s] @ C[n])                  [S, R]
  q = k = sum_n wqk[s,n] * (h[s] @ Eqk[n])           [S, D]
  v = sum_n wv[s,n] * (h[s] @ Ev[n])                 [S, D]
  causal attention per head (DH=64), y = attn @ W_O.T

Sharding (8 cores, B=4): 2 cores per batch. Both cores of a pair compute
the token-parallel prefix (h and the mixing tensors G) for their batch;
the expansion matmuls, attention and output projection are split by HEADS
(8 per core) — head selection is done purely by slicing the expansion
weights / W_O on the host, so all cores run one identical SPMD program.
Each core emits a partial y (its heads' contribution through W_O); the
host sums each pair's partials.

On-chip pipeline (bf16 matmuls, fp32 PSUM):
  x^T via DMA-xbar transposes -> stage A (token-major h_all [tok, n, r]) ->
  DVE combine using step-0 free-dim broadcast APs of the routing weights ->
  token-major G_qk/G_v -> DMA-transposed to nr-major -> B_qk -> d-major q^T
  (scaled by DH**-0.25 during the PSUM->SBUF copy); B_v -> token-major V65
  (64 V columns + one ones column per head: row 64 of the attention AV
  matmul then accumulates the softmax denominator l for free).
  Attention runs in the S^T orientation (k on partitions, q on the free
  dim): S^T tile = qaug^T @ rhs65, where contraction row 64 carries ones
  (lhsT side) and -m[q] (rhs side), m[q] = |q'[q]|^2 + DELTA computed
  on-chip (any per-q shift is softmax-invariant as long as it is applied
  consistently, which the matmul guarantees); exp on the scalar engine;
  causal 0/1 masks multiplied into diagonal tiles; afterwards 1/l is
  partition-broadcast by a step-0 DMA and applied during the mandatory
  PSUM->SBUF copy of U^T. Finally y_partial = attn^T.T @ W_O^T-slice.
"""

import numpy as np
import ml_dtypes

B, S, D, H, R, N = 4, 2048, 1024, 16, 64, 16
DH = D // H          # 64
NR = N * R           # 1024
P = 128
NCORES = 8
TCH = S // P         # 16 token chunks
DCH = D // P         # 8
NRCH = NR // P       # 8
HL = H // 2          # heads per core (8)
DL = HL * DH         # local d width (512)
DLCH = DL // P       # 4
DELTA = 32.0         # softmax shift margin: m[q] = |q'|^2 + DELTA
QSC = float(DH) ** -0.25

BF16 = ml_dtypes.bfloat16
_CACHE = {}


def _build_program():
    import concourse.bass as bass
    import concourse.tile as tile
    from concourse import bacc, mybir

    fp32 = mybir.dt.float32
    bf16 = mybir.dt.bfloat16
    AF = mybir.ActivationFunctionType
    ts = bass.ts

    nc = bacc.Bacc("TRN2", target_bir_lowering=False, debug=False,
                   num_devices=NCORES)

    x_d = nc.dram_tensor("x", [S, D], bf16, kind="ExternalInput").ap()
    w3_d = nc.dram_tensor("w3", [S, 3 * N], fp32, kind="ExternalInput").ap()
    C_d = nc.dram_tensor("C", [D, NR], bf16, kind="ExternalInput").ap()
    Eqk_d = nc.dram_tensor("Eqk", [NR, DL], bf16, kind="ExternalInput").ap()
    Ev_d = nc.dram_tensor("Ev", [NR, DL], bf16, kind="ExternalInput").ap()
    WOT_d = nc.dram_tensor("WOT", [DL, D], bf16, kind="ExternalInput").ap()
    y_d = nc.dram_tensor("y", [S, D], fp32, kind="ExternalOutput").ap()

    def apx(ap_in, extra):
        # append [step, count] dims (step-0 free-dim broadcasts)
        return bass.AP(tensor=ap_in.tensor, offset=ap_in.offset,
                       ap=[*ap_in.ap, *extra])

    with tile.TileContext(nc) as tc:
        with tc.tile_pool(name="persist", bufs=1) as persist:
            GqkT = persist.tile([P, NRCH, S], bf16)      # [nr, tok]
            GvT = persist.tile([P, NRCH, S], bf16)       # [nr, tok]
            qT = persist.tile([P, DLCH, S], bf16)        # scaled q^T
            V65 = persist.tile([P, TCH, HL * 65], bf16)  # V + ones cols
            attnT = persist.tile([P, DLCH, S], bf16)     # attn^T
            masks = persist.tile([P, 4, 512], bf16)      # diag causal masks
            onesBD = persist.tile([P, 2], bf16)          # block-diag ones

            nc.vector.memset(onesBD[:], 0.0)
            nc.vector.memset(onesBD[0:64, 0:1], 1.0)
            nc.vector.memset(onesBD[64:128, 1:2], 1.0)
            v65_ones = bass.AP(tensor=V65.tensor, offset=V65.offset + 64,
                               ap=[V65.ap[0], [HL * 65, TCH], [65, HL]])
            nc.vector.memset(v65_ones, 1.0)
            for j in range(4):
                mj = masks[:, j, :]
                nc.gpsimd.memset(mj, 1.0)
                # keep where (j*128 + k) <= ql  <=>  ql - k - j*128 >= 0
                nc.gpsimd.affine_select(
                    out=mj, in_=mj, compare_op=mybir.AluOpType.is_ge,
                    fill=0.0, base=-j * P, channel_multiplier=-1,
                    pattern=[[1, 512]])

            # ---------------- Phase A: x^T, h, G ----------------
            with tc.tile_pool(name="phA_w", bufs=1) as wpool, \
                 tc.tile_pool(name="phA", bufs=3) as pa, \
                 tc.tile_pool(name="phA_ps", bufs=2, space="PSUM") as pps:
                C_sb = wpool.tile([P, DCH, NR], bf16)
                nc.gpsimd.dma_start(
                    out=C_sb[:], in_=C_d.rearrange("(c p) n -> p c n", p=P))
                w_sb = wpool.tile([P, TCH, 3 * N], fp32)
                nc.gpsimd.dma_start(
                    out=w_sb[:], in_=w3_d.rearrange("(t p) n -> p t n", p=P))

                for t in range(TCH):
                    xb = pa.tile([P, D], bf16, tag="xb")
                    nc.gpsimd.dma_start(out=xb[:], in_=x_d[ts(t, P), :])
                    xT = pa.tile([P, DCH, P], bf16, tag="xT")
                    for c in range(DCH):
                        nc.sync.dma_start(out=xT[:, c, :], in_=xb[:, ts(c, P)],
                                          transpose=True)
                    psA0 = pps.tile([P, 512], fp32, tag="psA0")
                    psA1 = pps.tile([P, 512], fp32, tag="psA1")
                    for c in range(DCH):
                        nc.tensor.matmul(psA0[:], xT[:, c, :], C_sb[:, c, 0:512],
                                         start=(c == 0), stop=(c == DCH - 1))
                    for c in range(DCH):
                        nc.tensor.matmul(psA1[:], xT[:, c, :], C_sb[:, c, 512:NR],
                                         start=(c == 0), stop=(c == DCH - 1))
                    hw0 = pa.tile([P, 512], fp32, tag="hw0")
                    hw1 = pa.tile([P, 512], fp32, tag="hw1")
                    nc.vector.tensor_mul(
                        hw0[:].rearrange("p (m r) -> p m r", r=R),
                        psA0[:].rearrange("p (m r) -> p m r", r=R),
                        apx(w_sb[:, t, 0:8], [[0, R]]))
                    nc.vector.tensor_mul(
                        hw1[:].rearrange("p (m r) -> p m r", r=R),
                        psA1[:].rearrange("p (m r) -> p m r", r=R),
                        apx(w_sb[:, t, 8:16], [[0, R]]))
                    t1 = pa.tile([P, 512], fp32, tag="t1")
                    nc.vector.tensor_add(t1[:], hw0[:], hw1[:])
                    t2 = pa.tile([P, 256], fp32, tag="t2")
                    nc.vector.tensor_add(t2[:], t1[:, 0:256], t1[:, 256:512])
                    t3 = pa.tile([P, 128], fp32, tag="t3")
                    nc.vector.tensor_add(t3[:], t2[:, 0:128], t2[:, 128:256])
                    ht = pa.tile([P, R], fp32, tag="ht")
                    nc.vector.tensor_add(ht[:], t3[:, 0:64], t3[:, 64:128])
                    h_b = bass.AP(tensor=ht.tensor, offset=ht.offset,
                                  ap=[ht.ap[0], [0, N], [1, R]])
                    Gq = pa.tile([P, NR], bf16, tag="Gq")
                    nc.vector.tensor_mul(
                        Gq[:].rearrange("p (n r) -> p n r", r=R),
                        h_b, apx(w_sb[:, t, N:2 * N], [[0, R]]))
                    Gv = pa.tile([P, NR], bf16, tag="Gv")
                    nc.vector.tensor_mul(
                        Gv[:].rearrange("p (n r) -> p n r", r=R),
                        h_b, apx(w_sb[:, t, 2 * N:3 * N], [[0, R]]))
                    for c in range(NRCH):
                        nc.sync.dma_start(out=GqkT[:, c, ts(t, P)],
                                          in_=Gq[:, ts(c, P)], transpose=True)
                        nc.sync.dma_start(out=GvT[:, c, ts(t, P)],
                                          in_=Gv[:, ts(c, P)], transpose=True)

            # ---------------- Phase B: q^T and V65 ----------------
            with tc.tile_pool(name="phB_w", bufs=1) as wpool, \
                 tc.tile_pool(name="phB_ps", bufs=2, space="PSUM") as pps:
                Eqk_sb = wpool.tile([P, NRCH, DL], bf16)
                nc.gpsimd.dma_start(
                    out=Eqk_sb[:], in_=Eqk_d.rearrange("(c p) n -> p c n", p=P))
                Ev_sb = wpool.tile([P, NRCH, DL], bf16)
                nc.gpsimd.dma_start(
                    out=Ev_sb[:], in_=Ev_d.rearrange("(c p) n -> p c n", p=P))

                for s4 in range(4):
                    for dch in range(DLCH):
                        psB = pps.tile([P, 512], fp32, tag="psB")
                        for c in range(NRCH):
                            nc.tensor.matmul(psB[:], Eqk_sb[:, c, ts(dch, P)],
                                             GqkT[:, c, ts(s4, 512)],
                                             start=(c == 0), stop=(c == NRCH - 1))
                        nc.scalar.activation(qT[:, dch, ts(s4, 512)], psB[:],
                                             AF.Copy, scale=QSC)
                for t in range(TCH):
                    psV = pps.tile([P, 512], fp32, tag="psV")
                    for c in range(NRCH):
                        nc.tensor.matmul(psV[:], GvT[:, c, ts(t, P)],
                                         Ev_sb[:, c, :],
                                         start=(c == 0), stop=(c == NRCH - 1))
                    vdst = V65[:, t, :].rearrange("p (h u) -> p h u", u=65)
                    nc.vector.tensor_copy(
                        vdst[:, :, 0:64],
                        psV[:].rearrange("p (h r) -> p h r", r=64))

            # ---------------- Phase C: attention ----------------
            with tc.tile_pool(name="phC_q", bufs=2) as pq, \
                 tc.tile_pool(name="phC_s", bufs=2) as psml, \
                 tc.tile_pool(name="phC_e", bufs=3) as pe, \
                 tc.tile_pool(name="phC_ps", bufs=3, space="PSUM") as ppsS, \
                 tc.tile_pool(name="phC_psu", bufs=2, space="PSUM") as ppsU, \
                 tc.tile_pool(name="phC_psd", bufs=2, space="PSUM") as ppsD:
                for hc in range(DLCH):          # head pair = qT chunk
                    qaug = [None, None]
                    for half in (0, 1):
                        qa = pq.tile([65, S], bf16, tag=f"qaug{half}")
                        nc.sync.dma_start(
                            out=qa[0:64, :],
                            in_=qT[half * 64:half * 64 + 64, hc, :])
                        nc.vector.memset(qa[64:65, :], 1.0)
                        qaug[half] = qa
                    for s4 in range(4):
                        nkt = 4 * (s4 + 1)
                        qsl = ts(s4, 512)
                        sq2 = psml.tile([P, 512], bf16, tag="sq2")
                        nc.vector.tensor_mul(sq2[:], qT[:, hc, qsl],
                                             qT[:, hc, qsl])
                        psD2 = ppsD.tile([2, 512], fp32, tag="psD2")
                        nc.tensor.matmul(psD2[:], onesBD[:], sq2[:],
                                         start=True, stop=True)
                        negm2 = psml.tile([2, 512], bf16, tag="negm2")
                        nc.scalar.activation(negm2[:], psD2[:], AF.Copy,
                                             scale=-1.0, bias=-DELTA)
                        for half in (0, 1):
                            h = 2 * hc + half
                            prow = slice(half * 64, half * 64 + 64)
                            rhs65 = psml.tile([65, 512], bf16,
                                              tag=f"rhs{half}")
                            nc.sync.dma_start(out=rhs65[0:64, :],
                                              in_=qT[prow, hc, qsl])
                            nc.sync.dma_start(out=rhs65[64:65, :],
                                              in_=negm2[half:half + 1, :])
                            psU = ppsU.tile([65, 512], fp32, tag="psU")
                            for kt in range(nkt):
                                psS = ppsS.tile([P, 512], fp32, tag="psS")
                                nc.tensor.matmul(psS[:],
                                                 qaug[half][:, ts(kt, P)],
                                                 rhs65[:], start=True,
                                                 stop=True)
                                Et = pe.tile([P, 512], bf16, tag="Et")
                                nc.scalar.activation(Et[:], psS[:], AF.Exp)
                                j = kt - (nkt - 4)
                                if j >= 0:
                                    nc.vector.tensor_mul(Et[:], Et[:],
                                                         masks[:, j, :])
                                nc.tensor.matmul(
                                    psU[:], V65[:, kt, h * 65:h * 65 + 65],
                                    Et[:], start=(kt == 0),
                                    stop=(kt == nkt - 1))
                            l65 = psml.tile([65, 512], fp32, tag="l65")
                            nc.vector.reciprocal(l65[64:65, :], psU[64:65, :])
                            lrow0 = psml.tile([1, 512], fp32, tag="lrow0")
                            nc.sync.dma_start(out=lrow0[:], in_=l65[64:65, :])
                            RL = psml.tile([64, 512], fp32, tag="RL")
                            nc.gpsimd.partition_broadcast(RL[:], lrow0[:],
                                                          channels=64)
                            if half == 0:
                                nc.vector.tensor_mul(attnT[0:64, hc, qsl],
                                                     psU[0:64, :], RL[:])
                            else:
                                tmp = psml.tile([64, 512], bf16, tag="tmp")
                                nc.vector.tensor_mul(tmp[:], psU[0:64, :],
                                                     RL[:])
                                nc.sync.dma_start(
                                    out=attnT[64:128, hc, qsl], in_=tmp[:])

            # ---------------- Phase D: output projection ----------------
            with tc.tile_pool(name="phD_w", bufs=1) as wpool, \
                 tc.tile_pool(name="phD", bufs=3) as pd, \
                 tc.tile_pool(name="phD_ps", bufs=2, space="PSUM") as pps:
                WOT_sb = wpool.tile([P, DLCH, D], bf16)
                nc.gpsimd.dma_start(
                    out=WOT_sb[:], in_=WOT_d.rearrange("(c p) n -> p c n", p=P))
                for t in range(TCH):
                    psY0 = pps.tile([P, 512], fp32, tag="psY0")
                    psY1 = pps.tile([P, 512], fp32, tag="psY1")
                    for c in range(DLCH):
                        nc.tensor.matmul(psY0[:], attnT[:, c, ts(t, P)],
                                         WOT_sb[:, c, 0:512],
                                         start=(c == 0), stop=(c == DLCH - 1))
                    for c in range(DLCH):
                        nc.tensor.matmul(psY1[:], attnT[:, c, ts(t, P)],
                                         WOT_sb[:, c, 512:D],
                                         start=(c == 0), stop=(c == DLCH - 1))
                    yt = pd.tile([P, D], fp32, tag="yt")
                    nc.scalar.activation(yt[:, 0:512], psY0[:], AF.Copy)
                    nc.scalar.activation(yt[:, 512:D], psY1[:], AF.Copy)
                    nc.gpsimd.dma_start(out=y_d[ts(t, P), :], in_=yt[:])

    nc.compile()
    return nc


def _get_program():
    if "nc" not in _CACHE:
        _CACHE["nc"] = _build_program()
    return _CACHE["nc"]


def _prep_inputs(x, compress_weights, expand_weights_QK, expand_weights_V,
                 compress_neurons, expand_neurons_QK, expand_neurons_V, W_O):
    """Host-side sharding: in_maps for the 8 cores."""
    x = np.asarray(x, np.float32)
    C_flat = np.ascontiguousarray(
        np.asarray(compress_neurons, np.float32).transpose(1, 0, 2)
        .reshape(D, NR)).astype(BF16)
    Eqk_flat = np.asarray(expand_neurons_QK, np.float32).reshape(NR, D)
    Ev_flat = np.asarray(expand_neurons_V, np.float32).reshape(NR, D)
    WOT = np.ascontiguousarray(np.asarray(W_O, np.float32).T)  # [din, dout]

    in_maps = []
    for core in range(NCORES):
        b, half = core // 2, core % 2
        dsl = slice(half * DL, (half + 1) * DL)
        w3 = np.concatenate([
            np.asarray(compress_weights[b], np.float32),
            np.asarray(expand_weights_QK[b], np.float32),
            np.asarray(expand_weights_V[b], np.float32)], axis=1)
        in_maps.append({
            "x": x[b].astype(BF16),
            "w3": np.ascontiguousarray(w3),
            "C": C_flat,
            "Eqk": np.ascontiguousarray(Eqk_flat[:, dsl]).astype(BF16),
            "Ev": np.ascontiguousarray(Ev_flat[:, dsl]).astype(BF16),
            "WOT": np.ascontiguousarray(WOT[dsl, :]).astype(BF16),
        })
    return in_maps


def kernel(**inputs):
    from concourse import bass_utils
    nc = _get_program()
    in_maps = _prep_inputs(**inputs)
    res = bass_utils.run_bass_kernel_spmd(nc, in_maps,
                                          core_ids=list(range(NCORES)))
    out = np.empty((B, S, D), np.float32)
    for b in range(B):
        out[b] = res.results[2 * b]["y"] + res.results[2 * b + 1]["y"]
    return out


# revision 30
# speedup vs baseline: 1.0599x; 1.0599x over previous
"""Trainium2 Bass kernel for nn_NeuronCircuit_15229954031678 (moe_routing).

Math (per batch b, token s):
  h = sum_n cw[s,n] * (x[s] @ C[n])                  [S, R]
  q = k = sum_n wqk[s,n] * (h[s] @ Eqk[n])           [S, D]
  v = sum_n wv[s,n] * (h[s] @ Ev[n])                 [S, D]
  causal attention per head (DH=64), y = attn @ W_O.T

Sharding (8 cores, B=4): 2 cores per batch. Both cores of a pair compute
the token-parallel prefix (h, mixing tensors G) for their batch; the
expansion matmuls, attention and output projection are split by HEADS
(8 per core) — head selection is done purely by slicing the expansion
weights / W_O on the host, so all cores run one identical SPMD program.
Each core emits a partial y (its heads' contribution through W_O); the
host sums each pair's partials.

On-chip pipeline (bf16 matmuls, fp32 PSUM):
  host supplies x^T bf16 -> stage A (token-major h_all [tok, n, r]) ->
  DVE combine using step-0 free-dim broadcast APs of the routing weights ->
  token-major G_qk/G_v -> PE-transposed (via identity) to nr-major ->
  B_qk -> d-major q^T (scaled by DH**-0.25 on the PSUM->SBUF copy);
  B_v -> token-major V65 (64 V columns + a ones column per head: row 64 of
  the attention AV matmul then accumulates the softmax denominator l free).
  Attention runs in the S^T orientation (k on partitions, q on the free
  dim): S^T tile = qaug^T @ rhs65, where contraction row 64 carries ones
  (lhsT side) and -m[q] (rhs side), m[q] = |q'[q]|^2 + DELTA computed
  on-chip (any per-q shift is softmax-invariant if applied consistently,
  which the matmul guarantees); exp on the scalar engine over kt-tile
  pairs; causal 0/1 masks multiplied into diagonal tiles; 1/l is
  partition-broadcast (gpsimd) and applied during the mandatory
  PSUM->SBUF copy of U^T. Finally y_partial = attn^T.T @ W_O^T-slice.
"""

import numpy as np
import ml_dtypes

B, S, D, H, R, N = 4, 2048, 1024, 16, 64, 16
DH = D // H          # 64
NR = N * R           # 1024
P = 128
NCORES = 8
TCH = S // P         # 16 token chunks
DCH = D // P         # 8
NRCH = NR // P       # 8
HL = H // 2          # heads per core (8)
DL = HL * DH         # local d width (512)
DLCH = DL // P       # 4
DELTA = 32.0         # softmax shift margin: m[q] = |q'|^2 + DELTA
QSC = float(DH) ** -0.25

BF16 = ml_dtypes.bfloat16
_CACHE = {}


def _build_program(max_phase=4):
    import concourse.bass as bass
    import concourse.tile as tile
    from concourse import bacc, mybir
    from concourse.masks import make_identity

    fp32 = mybir.dt.float32
    bf16 = mybir.dt.bfloat16
    AF = mybir.ActivationFunctionType
    ts = bass.ts

    nc = bacc.Bacc("TRN2", target_bir_lowering=False, debug=False,
                   num_devices=NCORES)

    xT_d = nc.dram_tensor("xT", [D, S], bf16, kind="ExternalInput").ap()
    w3_d = nc.dram_tensor("w3", [S, 3 * N], fp32, kind="ExternalInput").ap()
    C_d = nc.dram_tensor("C", [D, NR], bf16, kind="ExternalInput").ap()
    Eqk_d = nc.dram_tensor("Eqk", [NR, DL], bf16, kind="ExternalInput").ap()
    Ev_d = nc.dram_tensor("Ev", [NR, DL], bf16, kind="ExternalInput").ap()
    WOT_d = nc.dram_tensor("WOT", [DL, D], bf16, kind="ExternalInput").ap()
    y_d = nc.dram_tensor("y", [S, D], fp32, kind="ExternalOutput").ap()

    def apx(ap_in, extra):
        # append [step, count] dims (step-0 free-dim broadcasts)
        return bass.AP(tensor=ap_in.tensor, offset=ap_in.offset,
                       ap=[*ap_in.ap, *extra])

    with tile.TileContext(nc) as tc:
        with tc.tile_pool(name="persist", bufs=1) as persist:
            GqkT = persist.tile([P, NRCH, S], bf16)      # [nr, tok]
            GvT = persist.tile([P, NRCH, S], bf16)       # [nr, tok]
            qT = persist.tile([P, DLCH, S], bf16)        # scaled q^T
            V65 = persist.tile([P, TCH, HL * 65], bf16)  # V + ones cols
            attnT = persist.tile([P, DLCH, S], bf16)     # attn^T
            tril = persist.tile([P, P], bf16)            # tril (incl diag)
            onesBD = persist.tile([P, 2], bf16)          # block-diag ones
            ident = persist.tile([P, P], bf16)           # PE transpose identity

            make_identity(nc, ident[:])
            nc.vector.memset(onesBD[:], 0.0)
            nc.vector.memset(onesBD[0:64, 0:1], 1.0)
            nc.vector.memset(onesBD[64:128, 1:2], 1.0)
            v65_ones = bass.AP(tensor=V65.tensor, offset=V65.offset + 64,
                               ap=[V65.ap[0], [HL * 65, TCH], [65, HL]])
            nc.vector.memset(v65_ones, 1.0)
            nc.gpsimd.memset(tril[:], 1.0)
            nc.gpsimd.affine_select(
                out=tril[:], in_=tril[:], compare_op=mybir.AluOpType.is_ge,
                fill=0.0, base=0, channel_multiplier=-1, pattern=[[1, P]])

            # ---------------- Phase A: h, G, G^T ----------------
            with tc.tile_pool(name="phA_w", bufs=1) as wpool, \
                 tc.tile_pool(name="phA", bufs=3) as pa, \
                 tc.tile_pool(name="phA_ps", bufs=2, space="PSUM") as pps, \
                 tc.tile_pool(name="phA_pst", bufs=4, space="PSUM") as ppst:
                C_sb = wpool.tile([P, DCH, NR], bf16)
                nc.gpsimd.dma_start(
                    out=C_sb[:], in_=C_d.rearrange("(c p) n -> p c n", p=P))
                w_sb = wpool.tile([P, TCH, 3 * N], fp32)
                nc.gpsimd.dma_start(
                    out=w_sb[:], in_=w3_d.rearrange("(t p) n -> p t n", p=P))
                xT_sb = wpool.tile([P, DCH, S], bf16)
                nc.gpsimd.dma_start(
                    out=xT_sb[:], in_=xT_d.rearrange("(c p) s -> p c s", p=P))

                for t in range(TCH):
                    psA0 = pps.tile([P, 512], fp32, tag="psA0")
                    psA1 = pps.tile([P, 512], fp32, tag="psA1")
                    for c in range(DCH):
                        nc.tensor.matmul(psA0[:], xT_sb[:, c, ts(t, P)],
                                         C_sb[:, c, 0:512],
                                         start=(c == 0), stop=(c == DCH - 1))
                    for c in range(DCH):
                        nc.tensor.matmul(psA1[:], xT_sb[:, c, ts(t, P)],
                                         C_sb[:, c, 512:NR],
                                         start=(c == 0), stop=(c == DCH - 1))
                    hw0 = pa.tile([P, 512], fp32, tag="hw0")
                    hw1 = pa.tile([P, 512], fp32, tag="hw1")
                    nc.vector.tensor_mul(
                        hw0[:].rearrange("p (m r) -> p m r", r=R),
                        psA0[:].rearrange("p (m r) -> p m r", r=R),
                        apx(w_sb[:, t, 0:8], [[0, R]]))
                    nc.vector.tensor_mul(
                        hw1[:].rearrange("p (m r) -> p m r", r=R),
                        psA1[:].rearrange("p (m r) -> p m r", r=R),
                        apx(w_sb[:, t, 8:16], [[0, R]]))
                    t1 = pa.tile([P, 512], fp32, tag="t1")
                    nc.vector.tensor_add(t1[:], hw0[:], hw1[:])
                    t2 = pa.tile([P, 256], fp32, tag="t2")
                    nc.vector.tensor_add(t2[:], t1[:, 0:256], t1[:, 256:512])
                    t3 = pa.tile([P, 128], fp32, tag="t3")
                    nc.vector.tensor_add(t3[:], t2[:, 0:128], t2[:, 128:256])
                    ht = pa.tile([P, R], fp32, tag="ht")
                    nc.vector.tensor_add(ht[:], t3[:, 0:64], t3[:, 64:128])
                    h_b = bass.AP(tensor=ht.tensor, offset=ht.offset,
                                  ap=[ht.ap[0], [0, N], [1, R]])
                    Gq = pa.tile([P, NR], bf16, tag="Gq")
                    nc.vector.tensor_mul(
                        Gq[:].rearrange("p (n r) -> p n r", r=R),
                        h_b, apx(w_sb[:, t, N:2 * N], [[0, R]]))
                    Gv = pa.tile([P, NR], bf16, tag="Gv")
                    nc.vector.tensor_mul(
                        Gv[:].rearrange("p (n r) -> p n r", r=R),
                        h_b, apx(w_sb[:, t, 2 * N:3 * N], [[0, R]]))
                    for c in range(NRCH):
                        pst = ppst.tile([P, P], bf16, tag="pst")
                        nc.tensor.transpose(pst[:], Gq[:, ts(c, P)], ident[:])
                        nc.scalar.activation(GqkT[:, c, ts(t, P)], pst[:], AF.Copy)
                        nc.sync.dma_start(out=GvT[:, c, ts(t, P)],
                                          in_=Gv[:, ts(c, P)], transpose=True)

            # ------- Phases B+C+D fused: expansion matmuls emitted just in
            # time inside the stripe-outer attention loop; output projection
            # for stripe s interleaved into stripe s+1 -------
            if max_phase >= 2:
                with tc.tile_pool(name="phC_w", bufs=1) as wpool2, \
                     tc.tile_pool(name="phC_q", bufs=1) as pq, \
                     tc.tile_pool(name="phC_s", bufs=2) as psml, \
                     tc.tile_pool(name="phC_e", bufs=3) as pe, \
                     tc.tile_pool(name="phC_y", bufs=2) as pd, \
                     tc.tile_pool(name="phC_ps", bufs=2, space="PSUM") as ppsS, \
                     tc.tile_pool(name="phC_psu", bufs=2, space="PSUM") as ppsU, \
                     tc.tile_pool(name="phC_psm", bufs=2, space="PSUM") as ppsM:
                    Eqk_sb = wpool2.tile([P, NRCH, DL], bf16)
                    nc.gpsimd.dma_start(
                        out=Eqk_sb[:],
                        in_=Eqk_d.rearrange("(c p) n -> p c n", p=P))
                    Ev_sb = wpool2.tile([P, NRCH, DL], bf16)
                    nc.gpsimd.dma_start(
                        out=Ev_sb[:],
                        in_=Ev_d.rearrange("(c p) n -> p c n", p=P))
                    WOT_sb = wpool2.tile([P, DLCH, D], bf16)
                    nc.gpsimd.dma_start(
                        out=WOT_sb[:],
                        in_=WOT_d.rearrange("(c p) n -> p c n", p=P))
                    qaug = {}

                    def emit_bv(t):
                        psV = ppsM.tile([P, 512], fp32, tag="misc")
                        for c in range(NRCH):
                            nc.tensor.matmul(psV[:], GvT[:, c, ts(t, P)],
                                             Ev_sb[:, c, :],
                                             start=(c == 0),
                                             stop=(c == NRCH - 1))
                        vdst = V65[:, t, :].rearrange("p (h u) -> p h u", u=65)
                        nc.vector.tensor_copy(
                            vdst[:, :, 0:64],
                            psV[:].rearrange("p (h r) -> p h r", r=64))

                    def emit_bqk(dch):
                        for s4 in range(4):
                            psB = ppsM.tile([P, 512], fp32, tag="misc")
                            for c in range(NRCH):
                                nc.tensor.matmul(
                                    psB[:], Eqk_sb[:, c, ts(dch, P)],
                                    GqkT[:, c, ts(s4, 512)],
                                    start=(c == 0), stop=(c == NRCH - 1))
                            nc.vector.tensor_scalar_mul(
                                qT[:, dch, ts(s4, 512)], psB[:], QSC)

                    def emit_outproj(tt, dh2):
                        psY = ppsM.tile([P, 512], fp32, tag="misc")
                        for c in range(DLCH):
                            nc.tensor.matmul(
                                psY[:], attnT[:, c, ts(tt, P)],
                                WOT_sb[:, c, dh2 * 512:dh2 * 512 + 512],
                                start=(c == 0), stop=(c == DLCH - 1))
                        yt = pd.tile([P, 512], fp32, tag="yt")
                        nc.vector.tensor_copy(yt[:], psY[:])
                        nc.gpsimd.dma_start(
                            out=y_d[ts(tt, P), dh2 * 512:dh2 * 512 + 512],
                            in_=yt[:])

                    # prologue: V chunks 0-3 and q^T chunk 0
                    JIT_B = globals().get("_JIT_B", True)
                    for t in range(4):
                        emit_bv(t)
                    emit_bqk(0)

                    if max_phase < 3 or not JIT_B:
                        for dch in range(1, DLCH):
                            emit_bqk(dch)
                        for t in range(4, TCH):
                            emit_bv(t)

                    for s4 in range(4 if max_phase >= 3 else 0):
                        nkt = 4 * (s4 + 1)
                        qsl = ts(s4, 512)
                        for hc in range(DLCH):      # head pair = qT chunk
                            # just-in-time expansion work for later stripes
                            if JIT_B and s4 == 0 and hc < 3:
                                emit_bqk(hc + 1)
                            if JIT_B and s4 < 3:
                                emit_bv(4 * (s4 + 1) + hc)
                            if max_phase >= 4 and s4 >= 1:
                                # interleave previous stripe's out-projection
                                p0 = 4 * (s4 - 1)
                                for k in (2 * hc, 2 * hc + 1):
                                    emit_outproj(p0 + k // 2, k % 2)
                            if s4 == 0:
                                for half in (0, 1):
                                    qa = pq.tile([65, S], bf16,
                                                 tag=f"qaug{hc}_{half}")
                                    if half == 0:
                                        nc.vector.tensor_copy(qa[0:64, :],
                                                              qT[0:64, hc, :])
                                    else:
                                        nc.scalar.dma_start(
                                            out=qa[0:64, :],
                                            in_=qT[64:128, hc, :])
                                    nc.vector.memset(qa[64:65, :], 1.0)
                                    qaug[(hc, half)] = qa
                            sq2 = psml.tile([P, 512], bf16, tag="sq2")
                            nc.vector.tensor_mul(sq2[:], qT[:, hc, qsl],
                                                 qT[:, hc, qsl])
                            psD2 = ppsM.tile([2, 512], fp32, tag="misc")
                            nc.tensor.matmul(psD2[:], onesBD[:], sq2[:],
                                             start=True, stop=True)
                            negm2 = psml.tile([2, 512], bf16, tag="negm2")
                            nc.scalar.activation(negm2[:], psD2[:], AF.Copy,
                                                 scale=-1.0, bias=-DELTA)
                            for half in (0, 1):
                                h = 2 * hc + half
                                rhs65 = psml.tile([65, 512], bf16,
                                                  tag=f"rhs{half}")
                                nc.vector.tensor_copy(
                                    rhs65[0:64, :], qaug[(hc, half)][0:64, qsl])
                                nc.sync.dma_start(
                                    out=rhs65[64:65, :],
                                    in_=negm2[half:half + 1, :])
                                psU = ppsU.tile([65, 512], fp32, tag="psU")
                                # tiles: (kt, q-offset, width, diag?) —
                                # diagonal tiles first (narrowed to the
                                # causal remainder), then full tiles; the
                                # last full tile carries stop over all cols.
                                if s4 == 0:
                                    tiles = [(kt, 0, 512, kt - (nkt - 4))
                                             for kt in range(nkt)]
                                else:
                                    tiles = [(nkt - 4 + j, j * P, 512 - j * P,
                                              j) for j in range(4)]
                                    tiles += [(kt, 0, 512, -1)
                                              for kt in range(nkt - 4)]
                                # pair tiles into one psS/exp op
                                pairs = [tiles[i:i + 2]
                                         for i in range(0, len(tiles), 2)]
                                for ip, pr in enumerate(pairs):
                                    wtot = sum(e[2] for e in pr)
                                    psS = ppsS.tile([P, 1024], fp32, tag="psS")
                                    a = 0
                                    for (kt, qo, w, j) in pr:
                                        nc.tensor.matmul(
                                            psS[:, a:a + w],
                                            qaug[(hc, half)][:, ts(kt, P)],
                                            rhs65[:, qo:qo + w],
                                            start=True, stop=True)
                                        a += w
                                    Et = pe.tile([P, 1024], bf16, tag="Et")
                                    nc.scalar.activation(Et[:, 0:wtot],
                                                         psS[:, 0:wtot],
                                                         AF.Exp)
                                    a = 0
                                    for ie, (kt, qo, w, j) in enumerate(pr):
                                        esl = Et[:, a:a + w]
                                        if j >= 0:
                                            if s4 == 0 and j > 0:
                                                nc.vector.memset(
                                                    Et[:, a:a + j * P], 0.0)
                                            do = a + (j * P if s4 == 0 else 0)
                                            nc.vector.tensor_mul(
                                                Et[:, do:do + P],
                                                Et[:, do:do + P], tril[:])
                                        first = (ip == 0 and ie == 0)
                                        last = (ip == len(pairs) - 1
                                                and ie == len(pr) - 1)
                                        nc.tensor.matmul(
                                            psU[:, qo:qo + w],
                                            V65[:, kt, h * 65:h * 65 + 65],
                                            esl, start=first, stop=last)
                                        a += w
                                l65 = psml.tile([65, 512], fp32, tag="l65")
                                nc.vector.reciprocal(l65[64:65, :],
                                                     psU[64:65, :])
                                Ustg = psml.tile([64, 512], fp32, tag="Ustg")
                                nc.vector.tensor_copy(Ustg[:], psU[0:64, :])
                                lrow0 = psml.tile([1, 512], fp32, tag="lrow0")
                                nc.sync.dma_start(out=lrow0[:],
                                                  in_=l65[64:65, :])
                                RL = psml.tile([64, 512], fp32, tag="RL")
                                nc.gpsimd.partition_broadcast(RL[:], lrow0[:],
                                                              channels=64)
                                if half == 0:
                                    nc.vector.tensor_mul(attnT[0:64, hc, qsl],
                                                         Ustg[:], RL[:])
                                else:
                                    tmp = psml.tile([64, 512], bf16, tag="tmp")
                                    nc.vector.tensor_mul(tmp[:], Ustg[:],
                                                         RL[:])
                                    nc.scalar.dma_start(
                                        out=attnT[64:128, hc, qsl], in_=tmp[:])
                    if max_phase >= 4:
                        for tt in range(12, 16):
                            for dh2 in (0, 1):
                                emit_outproj(tt, dh2)

    nc.compile()
    return nc


def _get_program():
    if "nc" not in _CACHE:
        _CACHE["nc"] = _build_program()
    return _CACHE["nc"]


def _prep_inputs(x, compress_weights, expand_weights_QK, expand_weights_V,
                 compress_neurons, expand_neurons_QK, expand_neurons_V, W_O):
    """Host-side sharding: in_maps for the 8 cores."""
    x = np.asarray(x, np.float32)
    C_flat = np.ascontiguousarray(
        np.asarray(compress_neurons, np.float32).transpose(1, 0, 2)
        .reshape(D, NR)).astype(BF16)
    Eqk_flat = np.asarray(expand_neurons_QK, np.float32).reshape(NR, D)
    Ev_flat = np.asarray(expand_neurons_V, np.float32).reshape(NR, D)
    WOT = np.ascontiguousarray(np.asarray(W_O, np.float32).T)  # [din, dout]

    in_maps = []
    for core in range(NCORES):
        b, half = core // 2, core % 2
        dsl = slice(half * DL, (half + 1) * DL)
        w3 = np.concatenate([
            np.asarray(compress_weights[b], np.float32),
            np.asarray(expand_weights_QK[b], np.float32),
            np.asarray(expand_weights_V[b], np.float32)], axis=1)
        in_maps.append({
            "xT": np.ascontiguousarray(x[b].T).astype(BF16),
            "w3": np.ascontiguousarray(w3),
            "C": C_flat,
            "Eqk": np.ascontiguousarray(Eqk_flat[:, dsl]).astype(BF16),
            "Ev": np.ascontiguousarray(Ev_flat[:, dsl]).astype(BF16),
            "WOT": np.ascontiguousarray(WOT[dsl, :]).astype(BF16),
        })
    return in_maps


def kernel(**inputs):
    from concourse import bass_utils
    nc = _get_program()
    in_maps = _prep_inputs(**inputs)
    res = bass_utils.run_bass_kernel_spmd(nc, in_maps,
                                          core_ids=list(range(NCORES)))
    out = np.empty((B, S, D), np.float32)
    for b in range(B):
        out[b] = res.results[2 * b]["y"] + res.results[2 * b + 1]["y"]
    return out
